# revision 3
# baseline (speedup 1.0000x reference)
"""Black-Scholes 'all' pricing on 8 Trainium2 NeuronCores (Bass/Tile).

kernel(S0, K, T, vt) -> [N, 4] float32 (call, put, digital_call, digital_put)
N = 8_388_608, sharded contiguously across 8 cores.
"""
import numpy as np

from bs_builder import build_bs
from concourse.bass_utils import run_bass_kernel_spmd

N = 8_388_608
NCORES = 8
P = 128
FD = N // NCORES // P  # 8192

_NC = None
LAST_EXEC_NS = None
LAST_TRACE_DIR = None
TRACE = False


def _get_nc():
    global _NC
    if _NC is None:
        _NC = build_bs(FD=FD, F=1024, G=2, P=P)
    return _NC


def kernel(S0, K, T, vt):
    global LAST_EXEC_NS, LAST_TRACE_DIR
    nc = _get_nc()
    shards = []
    arrs = {"s0": S0, "k": K, "t": T, "vt": vt}
    for i in range(NCORES):
        sl = slice(i * P * FD, (i + 1) * P * FD)
        shards.append({
            name: np.ascontiguousarray(np.asarray(a[sl], dtype=np.float32)
                                       .reshape(P, FD))
            for name, a in arrs.items()
        })
    kwargs = {}
    if TRACE:
        import tempfile
        LAST_TRACE_DIR = tempfile.mkdtemp(prefix="bs_trace_")
        kwargs = dict(trace=True, tmpdir=LAST_TRACE_DIR)
    res = run_bass_kernel_spmd(nc, shards, core_ids=list(range(NCORES)), **kwargs)
    LAST_EXEC_NS = res.exec_time_ns
    out = np.empty((N, 4), dtype=np.float32)
    for i in range(NCORES):
        sl = slice(i * P * FD, (i + 1) * P * FD)
        out[sl] = res.results[i]["out"].reshape(P * FD, 4)
    return out
